# revision 60
# baseline (speedup 1.0000x reference)
"""ChebConv (order-4) GNN layer on 8 Trainium2 NeuronCores.

Reference computation (fp32):
    T0 = x, T1 = G x, Tk = 2 G T{k-1} - T{k-2}
    out = sum_k Tk @ W[k]          # [N, F] with N=10000, F=32

Strategy:
  * Rewrite in the power basis: y0 = x, yk = G y{k-1},
      out = sum_k yk @ Wp[k]  with
      Wp = [W0 - W2, W1 - 3 W3, 2 W2, 4 W3]   (exact modulo fp reassociation)
    so each hop is a bare matmul against G (no 2*/- epilogue).
  * Row-shard G over 8 cores (1280 padded rows each; pad N 10000 ->
    10240).  Everything runs in plain bf16 (fp32 PSUM accumulation):
    measured end-to-end relative error ~3.4e-3 against the fp64
    oracle, inside the 2e-2 gate, and one bf16 pass per hop is 3x
    less PE streaming and 2x less HBM than an fp32-accurate scheme.
  * The ENTIRE per-core G^T block (~202KB/partition bf16) is pinned
    in SBUF: G is read from HBM exactly once, during hop 1.  Hops 2
    and 3 run pure PE with zero G DMA, which also keeps the shared
    HWDGE completion-semaphore rotation free of slow epochs (those
    stalled the collective staging DMAs by 30-50us in streaming
    variants).  To fit, everything non-G is trimmed to ~13KB per
    partition: y^T lives per-sweep in bf16, the k=0 term (x @ Wp0)
    is added by the HOST after gather, per-hop output partials ship
    to DRAM in bf16 for host summation (no on-chip cross-hop
    accumulator; PSUM groups held open across hops wedge the PE),
    and part 2's v tile is reused in place (its gather completes at
    hop end anyway).
  * Each hop runs 3 sweeps over column chunks (512, 384, 384) of
    yk^T.  Per sweep: matmuls (lhsT=v[j-chunk] [128,32], rhs=G^T
    tile [128,l]) accumulate the sweep's [32,l] chunk of yk^T over
    all 79 valid 128-row j-chunks in one open PSUM accumulation
    group (the all-padding j-chunk 79 is skipped).
  * The host pre-lays G^T out partition-major in the kernel's j
    consumption order: g_i[p, pos*l + c] = G^T[jorder[pos]*128 + p,
    s + c].  Hop-1 fills are plain 2D DMAs with 6-8KB contiguous
    per-partition lines (8 j-chunks per descriptor), near peak DMA
    rate, consumed incrementally by the PE.
  * After each sweep (except in the last hop), its rows are
    PE-transposed into natural m-chunk layout, cast bf16, and
    all-gathered in a partial collective (DRAM bounce) that overlaps
    the remaining sweeps.  Queue placement is load-bearing: cc_in
    staging rides scalar (HWDGE; its rotation holds only fast
    epochs), v reloads ride sync (idle after hop 1), and the output
    partials ride SWDGE so the y_t bounce never waits on the HWDGE
    rotation.  j-chunks are consumed in gather-firing order so each
    hop starts on columns whose gather finished first.  The
    remaining stalls (~25us) are the collectives stream's cold-start
    barrier (33-55us, runtime-internal) gating the first gather, and
    ~8-18us per collective op thereafter.
  * Output: three per-hop partials [32, 1280] bf16 per core; the
    host concatenates, transposes, sums, drops padding and adds the
    k=0 term in fp32.
"""

import sys

if "/opt/trn_rl_repo" not in sys.path:
    sys.path.insert(0, "/opt/trn_rl_repo")

import numpy as np

N = 10000
F = 32
ORDER = 4
NCORES = 8
P = 128
NP = 10240  # padded node count: divisible by NCORES * P
RPC = NP // NCORES  # rows per core (1280)
JC = NP // P  # global 128-row chunks (80); the last is all padding
JCV = JC - 1  # valid chunks
MC = RPC // P  # local 128-row chunks per core (10)

# column chunks of the per-core output slice: (start, len)
FCHUNKS = [(0, 512), (512, 384), (896, 384)]
# per-part m-chunk geometry: part i covers m-chunks [m0, m0+nm)
PARTS = [(0, 4), (4, 3), (7, 3)]
SWEEP_ORDER = [0, 1, 2]
GRP = 8  # j-chunks per hop-1 fill descriptor

_CACHE = {}


def _jorder():
    """j-chunk consumption order: grouped by producing part in sweep
    order (= gather firing order), skipping the all-padding chunk."""
    order = []
    for i in SWEEP_ORDER:
        m0, nm = PARTS[i]
        for c in range(NCORES):
            for mm in range(nm):
                j = c * MC + m0 + mm
                if j != JC - 1:
                    order.append(j)
    return order


def _build(np_total, ncores):
    from concourse import bacc, masks, mybir, tile

    rpc = np_total // ncores
    jc = np_total // P
    mc = rpc // P
    f32 = mybir.dt.float32
    bf16 = mybir.dt.bfloat16
    jcv = jc - 1

    nc = bacc.Bacc(
        "TRN2", target_bir_lowering=False, debug=False, num_devices=ncores
    )
    # per-sweep G^T blocks, host-laid-out partition-major in jorder:
    # g_i[p, pos*l + c] = G^T[jorder[pos]*P + p, s + c]
    gs = [
        nc.dram_tensor(f"g{i}", [P, jcv * l], bf16, kind="ExternalInput").ap()
        for i, (s, l) in enumerate(FCHUNKS)
    ]
    vcols = [ncores * nm * F for (m0, nm) in PARTS]
    # x in per-part v layout: block col (c*nm + ml)*F + f
    #   = padded x row (c*mc + m0 + ml)*P + p
    xtb = nc.dram_tensor("xtb", [P, sum(vcols)], bf16, kind="ExternalInput").ap()
    wp = nc.dram_tensor("wp", [F, (ORDER - 1) * F], bf16, kind="ExternalInput").ap()
    # per-hop partial outputs (y_k @ Wp_k)^T; the host sums them
    outs_t = [
        nc.dram_tensor(f"out{k}T", [F, rpc], bf16, kind="ExternalOutput").ap()
        for k in range(1, ORDER)
    ]

    jorder = _jorder()
    assert len(jorder) == jcv
    groups = [(g0, min(GRP, jcv - g0)) for g0 in range(0, jcv, GRP)]

    def part_of(m):
        for i, (m0, nm) in enumerate(PARTS):
            if m0 <= m < m0 + nm:
                return i
        raise AssertionError

    with tile.TileContext(nc) as tc:
        with (
            tc.tile_pool(name="const", bufs=1) as constp,
            tc.tile_pool(name="vp", bufs=2) as vp,
            tc.tile_pool(name="sb", bufs=1) as sb,
            tc.tile_pool(name="ps_hop", bufs=1, space="PSUM") as ps_hop,
            tc.tile_pool(name="ps_tp", bufs=2, space="PSUM") as ps_tp,
            tc.tile_pool(name="ps_w", bufs=2, space="PSUM") as ps_w,
            tc.tile_pool(name="dram", bufs=2, space="DRAM") as dram,
        ):
            ident = constp.tile([F, F], bf16)
            masks.make_identity(nc, ident[:])
            w_sb = constp.tile([F, (ORDER - 1) * F], bf16)
            nc.scalar.dma_start(w_sb[:], wp)
            pins = [
                constp.tile([P, jcv * l], bf16, name=f"pin{i}")
                for i, (s, l) in enumerate(FCHUNKS)
            ]
            # all hop-1 G fills issue up front so the sync queue never
            # stalls fill issue behind a cc_in staging wait; the PE's
            # matmuls consume each slice as its descriptor lands
            for i in SWEEP_ORDER:
                l = FCHUNKS[i][1]
                for g0, gl in groups:
                    nc.sync.dma_start(
                        pins[i][:, g0 * l : (g0 + gl) * l],
                        gs[i][:, g0 * l : (g0 + gl) * l],
                    )

            # v holds y_{k-1} in bf16, one tile per fc part so next-hop
            # matmuls only depend on the partial gather that produced
            # their columns
            v_parts = []
            off = 0
            for i, w_ in enumerate(vcols):
                if i < 2:
                    vt = vp.tile([P, w_], bf16, tag=f"v{i}", name=f"v{i}")
                else:
                    vt = constp.tile([P, w_], bf16, name=f"v{i}")
                nc.scalar.dma_start(vt[:], xtb[:, off : off + w_])
                off += w_
                v_parts.append(vt)

            def v_of(vps, j):
                c, m = j // mc, j % mc
                i = part_of(m)
                m0, nm = PARTS[i]
                col = (c * nm + (m - m0)) * F
                return vps[i][:, col : col + F]

            for k in range(1, ORDER):
                v_cur = v_parts
                if k < ORDER - 1:
                    # parts 0/1 double-buffer so their reloads can land
                    # mid-hop; part 2's gather only completes at hop end
                    # anyway, so its reload overwrites the tile in place
                    v_next = [
                        vp.tile([P, w_], bf16, tag=f"v{i}", name=f"vn{i}_{k}")
                        if i < 2
                        else v_parts[2]
                        for i, w_ in enumerate(vcols)
                    ]
                # hop: y_k^T = (G @ y_{k-1})^T, one sweep per fc chunk so
                # partial all-gathers overlap the remaining sweeps
                for i in SWEEP_ORDER:
                    s, l = FCHUNKS[i]
                    y_t = sb.tile([F, l], bf16, tag="yT", name="yT")
                    hp = ps_hop.tile([F, l], f32, tag=f"hop{i}", name=f"hp{i}")
                    for pos in range(jcv):
                        nc.tensor.matmul(
                            hp[:],
                            lhsT=v_of(v_cur, jorder[pos]),
                            rhs=pins[i][:, pos * l : (pos + 1) * l],
                            start=(pos == 0),
                            stop=(pos == jcv - 1),
                        )
                    # sweep epilogue: copy out, Wp contribution
                    nc.vector.tensor_copy(y_t[:], hp[:])
                    pw = ps_w.tile([F, l], f32, tag="pw", name="pw")
                    nc.tensor.matmul(
                        pw[:], lhsT=w_sb[:, (k - 1) * F : k * F],
                        rhs=y_t[:], start=True, stop=True,
                    )
                    if k < ORDER - 1:
                        # transpose this sweep's rows to natural layout,
                        # cast bf16, partial all-gather
                        m0, nm = PARTS[i]
                        stage = sb.tile(
                            [P, 4 * F], bf16, tag="stage", name="stage"
                        )
                        for mm in range(nm):
                            tp = ps_tp.tile([P, F], bf16, tag="tp", name="tp")
                            nc.tensor.transpose(
                                tp[:], y_t[:, mm * P : (mm + 1) * P],
                                ident[:],
                            )
                            nc.vector.tensor_copy(
                                stage[:, mm * F : (mm + 1) * F], tp[:]
                            )
                        cc_in = dram.tile(
                            [P, nm * F], bf16, tag=f"ccin{i}",
                            name=f"ccin{i}",
                        )
                        cc_out = dram.tile(
                            [ncores * P, nm * F], bf16, tag=f"ccout{i}",
                            name=f"ccout{i}",
                        )
                        # sync queue: its rotation holds only fast epochs
                        # (fills + stagings), so the collective trigger
                        # never inherits a gather-gated reload epoch
                        nc.sync.dma_start(cc_in[:], stage[:, 0 : nm * F])
                        nc.gpsimd.collective_compute(
                            "AllGather",
                            mybir.AluOpType.bypass,
                            replica_groups=[list(range(ncores))],
                            ins=[cc_in.opt()],
                            outs=[cc_out.opt()],
                        )
                        # scalar queue: idle after startup, so the
                        # gather-gated reloads only serialize among
                        # themselves (gathers complete in order anyway)
                        nc.scalar.dma_start(
                            v_next[i][:].rearrange("p (c m) -> p c m", c=ncores),
                            cc_out[:].rearrange("(c p) m -> p c m", p=P),
                        )
                    # fold this hop's output partial back through y_t
                    # (free once the transposes are done), bf16, ship it
                    # on SWDGE so it stays out of the HWDGE semaphore
                    # rotation that the reloads ride
                    nc.vector.tensor_copy(y_t[:], pw[:])
                    nc.gpsimd.dma_start(outs_t[k - 1][:, s : s + l], y_t[:])
                if k < ORDER - 1:
                    v_parts = v_next

    nc.compile()
    return nc


def get_nc(np_total=NP, ncores=NCORES):
    key = (np_total, ncores)
    if key not in _CACHE:
        _CACHE[key] = _build(np_total, ncores)
    return _CACHE[key]


def prep_inputs(x, gso, weight, np_total=NP, ncores=NCORES):
    """Host-side shard prep. Returns in_maps for run_bass_kernel_spmd."""
    import ml_dtypes

    bf = ml_dtypes.bfloat16
    n = x.shape[0]
    rpc = np_total // ncores
    jc = np_total // P
    jcv = jc - 1
    mc = rpc // P

    x = np.asarray(x, dtype=np.float32)
    gso = np.asarray(gso, dtype=np.float32)
    weight = np.asarray(weight, dtype=np.float32)

    # power-basis weights for k>=1; k=0 is added on the host
    wp = np.concatenate(
        [
            weight[1] - 3.0 * weight[3],
            2.0 * weight[2],
            4.0 * weight[3],
        ],
        axis=1,
    ).astype(bf)  # [F, (ORDER-1)*F]

    xpad = np.zeros((np_total, F), dtype=np.float32)
    xpad[:n] = x

    gpad = np.zeros((np_total, np_total), dtype=np.float32)
    gpad[:n, :n] = gso
    g16 = gpad.astype(bf)

    # x (bf16) in the per-part v layout
    x16 = xpad.astype(bf)

    def part_x(m0, nm):
        return np.ascontiguousarray(
            x16.reshape(ncores, mc, P, F)[:, m0 : m0 + nm]
            .transpose(2, 0, 1, 3)
            .reshape(P, ncores * nm * F)
        )

    xtb = np.ascontiguousarray(
        np.concatenate([part_x(m0, nm) for (m0, nm) in PARTS], axis=1)
    )

    jorder = np.asarray(_jorder())
    in_maps = []
    for c in range(ncores):
        rows = slice(c * rpc, (c + 1) * rpc)
        gt_c = g16[rows, :].T  # [np_total, rpc] bf16 view
        m = {"xtb": xtb, "wp": wp}
        for i, (s, l) in enumerate(FCHUNKS):
            # partition-major supertile layout in jorder
            chunk = np.ascontiguousarray(gt_c[:, s : s + l]).reshape(jc, P, l)
            m[f"g{i}"] = np.ascontiguousarray(
                chunk[jorder].transpose(1, 0, 2).reshape(P, jcv * l)
            )
        in_maps.append(m)
    return in_maps


def assemble_output(results, x, weight, n=N, ncores=NCORES):
    out_t = sum(
        np.concatenate(
            [results[c][f"out{k}T"] for c in range(ncores)], axis=1
        ).astype(np.float32)
        for k in range(1, ORDER)
    )
    out = np.ascontiguousarray(out_t.T[:n])
    # k=0 term, host-side in fp32
    wp0 = (weight[0] - weight[2]).astype(np.float32)
    out += np.asarray(x, dtype=np.float32) @ wp0
    return out


def kernel(x, gso, weight):
    import time

    from concourse import bass_utils

    nc = get_nc()
    in_maps = prep_inputs(x, gso, weight)
    last_err = None
    for attempt in range(3):
        try:
            res = bass_utils.run_bass_kernel_spmd(
                nc, in_maps, core_ids=list(range(NCORES))
            )
            return assemble_output(res.results, x, weight)
        except Exception as e:  # transient device wedge: retry
            last_err = e
            time.sleep(5.0 * (attempt + 1))
    raise last_err


# revision 63
# speedup vs baseline: 1.1012x; 1.1012x over previous
"""ChebConv (order-4) GNN layer on 8 Trainium2 NeuronCores.

Reference computation (fp32):
    T0 = x, T1 = G x, Tk = 2 G T{k-1} - T{k-2}
    out = sum_k Tk @ W[k]          # [N, F] with N=10000, F=32

Strategy:
  * Rewrite in the power basis: y0 = x, yk = G y{k-1},
      out = sum_k yk @ Wp[k]  with
      Wp = [W0 - W2, W1 - 3 W3, 2 W2, 4 W3]   (exact modulo fp reassociation)
    so each hop is a bare matmul against G (no 2*/- epilogue).
  * Row-shard G over 8 cores (1280 padded rows each; pad N 10000 ->
    10240).  Everything runs in plain bf16 (fp32 PSUM accumulation):
    measured end-to-end relative error ~3.4e-3 against the fp64
    oracle, inside the 2e-2 gate, and one bf16 pass per hop is 3x
    less PE streaming and 2x less HBM than an fp32-accurate scheme.
  * The ENTIRE per-core G^T block (~202KB/partition bf16) is pinned
    in SBUF: G is read from HBM exactly once, during hop 1.  Hops 2
    and 3 run pure PE with zero G DMA, which also keeps the shared
    HWDGE completion-semaphore rotation free of slow epochs (those
    stalled the collective staging DMAs by 30-50us in streaming
    variants).  To fit, everything non-G is trimmed to ~13KB per
    partition: y^T lives per-sweep in bf16, the k=0 term (x @ Wp0)
    is added by the HOST after gather, per-hop output partials ship
    to DRAM in bf16 for host summation (no on-chip cross-hop
    accumulator; PSUM groups held open across hops wedge the PE),
    and part 2's v tile is reused in place (its gather completes at
    hop end anyway).
  * Each hop runs 3 sweeps over column chunks (512, 384, 384) of
    yk^T.  Per sweep: matmuls (lhsT=v[j-chunk] [128,32], rhs=G^T
    tile [128,l]) accumulate the sweep's [32,l] chunk of yk^T over
    all 79 valid 128-row j-chunks in one open PSUM accumulation
    group (the all-padding j-chunk 79 is skipped).
  * The host pre-lays G^T out partition-major in the kernel's j
    consumption order: g_i[p, pos*l + c] = G^T[jorder[pos]*128 + p,
    s + c].  Hop-1 fills are plain 2D DMAs with 6-8KB contiguous
    per-partition lines (8 j-chunks per descriptor), near peak DMA
    rate, consumed incrementally by the PE.
  * After each sweep (except in the last hop), its rows are
    PE-transposed into natural m-chunk layout, cast bf16, and
    all-gathered in a partial collective (DRAM bounce) that overlaps
    the remaining sweeps.  Queue placement is load-bearing: cc_in
    staging rides scalar (HWDGE; its rotation holds only fast
    epochs), v reloads ride sync (idle after hop 1), and the output
    partials ride SWDGE so the y_t bounce never waits on the HWDGE
    rotation.  j-chunks are consumed in gather-firing order so each
    hop starts on columns whose gather finished first.  The
    remaining stalls (~25us) are the collectives stream's cold-start
    barrier (33-55us, runtime-internal) gating the first gather, and
    ~8-18us per collective op thereafter.
  * Output: three per-hop partials [32, 1280] bf16 per core; the
    host concatenates, transposes, sums, drops padding and adds the
    k=0 term in fp32.
"""

import sys

if "/opt/trn_rl_repo" not in sys.path:
    sys.path.insert(0, "/opt/trn_rl_repo")

import numpy as np

N = 10000
F = 32
ORDER = 4
NCORES = 8
P = 128
NP = 10240  # padded node count: divisible by NCORES * P
RPC = NP // NCORES  # rows per core (1280)
JC = NP // P  # global 128-row chunks (80); the last is all padding
JCV = JC - 1  # valid chunks
MC = RPC // P  # local 128-row chunks per core (10)

# column chunks of the per-core output slice: (start, len)
FCHUNKS = [(0, 512), (512, 384), (896, 384)]
# per-part m-chunk geometry: part i covers m-chunks [m0, m0+nm)
PARTS = [(0, 4), (4, 3), (7, 3)]
SWEEP_ORDER = [0, 1, 2]
GRP = 8  # j-chunks per hop-1 fill descriptor

_CACHE = {}


def _jorder():
    """j-chunk consumption order: grouped by producing part in sweep
    order (= gather firing order), skipping the all-padding chunk."""
    order = []
    for i in SWEEP_ORDER:
        m0, nm = PARTS[i]
        for c in range(NCORES):
            for mm in range(nm):
                j = c * MC + m0 + mm
                if j != JC - 1:
                    order.append(j)
    return order


def _build(np_total, ncores):
    from concourse import bacc, masks, mybir, tile

    rpc = np_total // ncores
    jc = np_total // P
    mc = rpc // P
    f32 = mybir.dt.float32
    bf16 = mybir.dt.bfloat16
    jcv = jc - 1

    nc = bacc.Bacc(
        "TRN2", target_bir_lowering=False, debug=False, num_devices=ncores
    )
    # per-sweep G^T blocks, host-laid-out partition-major in jorder:
    # g_i[p, pos*l + c] = G^T[jorder[pos]*P + p, s + c]
    gs = [
        nc.dram_tensor(f"g{i}", [P, jcv * l], bf16, kind="ExternalInput").ap()
        for i, (s, l) in enumerate(FCHUNKS)
    ]
    vcols = [ncores * nm * F for (m0, nm) in PARTS]
    # x in per-part v layout: block col (c*nm + ml)*F + f
    #   = padded x row (c*mc + m0 + ml)*P + p
    xtb = nc.dram_tensor("xtb", [P, sum(vcols)], bf16, kind="ExternalInput").ap()
    wp = nc.dram_tensor("wp", [F, (ORDER - 1) * F], bf16, kind="ExternalInput").ap()
    # per-hop partial outputs (y_k @ Wp_k)^T; the host sums them
    outs_t = [
        nc.dram_tensor(f"out{k}T", [F, rpc], bf16, kind="ExternalOutput").ap()
        for k in range(1, ORDER)
    ]

    jorder = _jorder()
    assert len(jorder) == jcv
    groups = [(g0, min(GRP, jcv - g0)) for g0 in range(0, jcv, GRP)]

    def part_of(m):
        for i, (m0, nm) in enumerate(PARTS):
            if m0 <= m < m0 + nm:
                return i
        raise AssertionError

    with tile.TileContext(nc) as tc:
        with (
            tc.tile_pool(name="const", bufs=1) as constp,
            tc.tile_pool(name="vp", bufs=2) as vp,
            tc.tile_pool(name="sb", bufs=1) as sb,
            tc.tile_pool(name="ps_hop", bufs=1, space="PSUM") as ps_hop,
            tc.tile_pool(name="ps_tp", bufs=2, space="PSUM") as ps_tp,
            tc.tile_pool(name="ps_w", bufs=2, space="PSUM") as ps_w,
            tc.tile_pool(name="dram", bufs=2, space="DRAM") as dram,
        ):
            ident = constp.tile([F, F], bf16)
            masks.make_identity(nc, ident[:])
            w_sb = constp.tile([F, (ORDER - 1) * F], bf16)
            nc.scalar.dma_start(w_sb[:], wp)
            pins = [
                constp.tile([P, jcv * l], bf16, name=f"pin{i}")
                for i, (s, l) in enumerate(FCHUNKS)
            ]
            # all hop-1 G fills issue up front so the sync queue never
            # stalls fill issue behind a cc_in staging wait; the PE's
            # matmuls consume each slice as its descriptor lands
            for i in SWEEP_ORDER:
                l = FCHUNKS[i][1]
                for g0, gl in groups:
                    nc.sync.dma_start(
                        pins[i][:, g0 * l : (g0 + gl) * l],
                        gs[i][:, g0 * l : (g0 + gl) * l],
                    )

            # v holds y_{k-1} in bf16, one tile per fc part so next-hop
            # matmuls only depend on the partial gather that produced
            # their columns
            v_parts = []
            off = 0
            for i, w_ in enumerate(vcols):
                if i < 2:
                    vt = vp.tile([P, w_], bf16, tag=f"v{i}", name=f"v{i}")
                else:
                    vt = constp.tile([P, w_], bf16, name=f"v{i}")
                nc.scalar.dma_start(vt[:], xtb[:, off : off + w_])
                off += w_
                v_parts.append(vt)

            def v_of(vps, j):
                c, m = j // mc, j % mc
                i = part_of(m)
                m0, nm = PARTS[i]
                col = (c * nm + (m - m0)) * F
                return vps[i][:, col : col + F]

            for k in range(1, ORDER):
                v_cur = v_parts
                if k < ORDER - 1:
                    # parts 0/1 double-buffer so their reloads can land
                    # mid-hop; part 2's gather only completes at hop end
                    # anyway, so its reload overwrites the tile in place
                    v_next = [
                        vp.tile([P, w_], bf16, tag=f"v{i}", name=f"vn{i}_{k}")
                        if i < 2
                        else v_parts[2]
                        for i, w_ in enumerate(vcols)
                    ]
                # hop: y_k^T = (G @ y_{k-1})^T, one sweep per fc chunk so
                # partial all-gathers overlap the remaining sweeps
                for i in SWEEP_ORDER:
                    s, l = FCHUNKS[i]
                    y_t = sb.tile([F, l], bf16, tag="yT", name="yT")
                    hp = ps_hop.tile([F, l], f32, tag=f"hop{i}", name=f"hp{i}")
                    for pos in range(jcv):
                        nc.tensor.matmul(
                            hp[:],
                            lhsT=v_of(v_cur, jorder[pos]),
                            rhs=pins[i][:, pos * l : (pos + 1) * l],
                            start=(pos == 0),
                            stop=(pos == jcv - 1),
                        )
                    # sweep epilogue: copy out, Wp contribution
                    nc.vector.tensor_copy(y_t[:], hp[:])
                    pw = ps_w.tile([F, l], f32, tag="pw", name="pw")
                    nc.tensor.matmul(
                        pw[:], lhsT=w_sb[:, (k - 1) * F : k * F],
                        rhs=y_t[:], start=True, stop=True,
                    )
                    if k < ORDER - 1:
                        # transpose this sweep's rows to natural layout,
                        # cast bf16, partial all-gather
                        m0, nm = PARTS[i]
                        stage = sb.tile(
                            [P, 4 * F], bf16, tag="stage", name="stage"
                        )
                        for mm in range(nm):
                            tp = ps_tp.tile([P, F], bf16, tag="tp", name="tp")
                            nc.tensor.transpose(
                                tp[:], y_t[:, mm * P : (mm + 1) * P],
                                ident[:],
                            )
                            nc.vector.tensor_copy(
                                stage[:, mm * F : (mm + 1) * F], tp[:]
                            )
                        cc_in = dram.tile(
                            [P, nm * F], bf16, tag=f"ccin{i}",
                            name=f"ccin{i}",
                        )
                        cc_out = dram.tile(
                            [ncores * P, nm * F], bf16, tag=f"ccout{i}",
                            name=f"ccout{i}",
                        )
                        # scalar: an otherwise-idle HWDGE ring, so the
                        # staging transfer starts immediately (the sync
                        # ring would queue it behind all hop-1 G fills)
                        nc.scalar.dma_start(cc_in[:], stage[:, 0 : nm * F])
                        nc.gpsimd.collective_compute(
                            "AllGather",
                            mybir.AluOpType.bypass,
                            replica_groups=[list(range(ncores))],
                            ins=[cc_in.opt()],
                            outs=[cc_out.opt()],
                        )
                        # SWDGE: keeps the gather-gated reload out of the
                        # global HWDGE semaphore rotation (it poisoned
                        # later cc_in stagings there).  Its generation
                        # blocks the NEXT trigger only until this gather
                        # completes, which is when the serial CC stream
                        # frees up anyway.
                        nc.gpsimd.dma_start(
                            v_next[i][:].rearrange("p (c m) -> p c m", c=ncores),
                            cc_out[:].rearrange("(c p) m -> p c m", p=P),
                        )
                    # fold this hop's output partial back through y_t
                    # (free once the transposes are done), bf16, ship it
                    # on scalar HWDGE: it must NOT ride SWDGE, where it
                    # would queue behind pending collectives and stall
                    # the next sweep's y_t reuse
                    nc.vector.tensor_copy(y_t[:], pw[:])
                    nc.scalar.dma_start(outs_t[k - 1][:, s : s + l], y_t[:])
                if k < ORDER - 1:
                    v_parts = v_next

    nc.compile()
    return nc


def get_nc(np_total=NP, ncores=NCORES):
    key = (np_total, ncores)
    if key not in _CACHE:
        _CACHE[key] = _build(np_total, ncores)
    return _CACHE[key]


def prep_inputs(x, gso, weight, np_total=NP, ncores=NCORES):
    """Host-side shard prep. Returns in_maps for run_bass_kernel_spmd."""
    import ml_dtypes

    bf = ml_dtypes.bfloat16
    n = x.shape[0]
    rpc = np_total // ncores
    jc = np_total // P
    jcv = jc - 1
    mc = rpc // P

    x = np.asarray(x, dtype=np.float32)
    gso = np.asarray(gso, dtype=np.float32)
    weight = np.asarray(weight, dtype=np.float32)

    # power-basis weights for k>=1; k=0 is added on the host
    wp = np.concatenate(
        [
            weight[1] - 3.0 * weight[3],
            2.0 * weight[2],
            4.0 * weight[3],
        ],
        axis=1,
    ).astype(bf)  # [F, (ORDER-1)*F]

    xpad = np.zeros((np_total, F), dtype=np.float32)
    xpad[:n] = x

    gpad = np.zeros((np_total, np_total), dtype=np.float32)
    gpad[:n, :n] = gso
    g16 = gpad.astype(bf)

    # x (bf16) in the per-part v layout
    x16 = xpad.astype(bf)

    def part_x(m0, nm):
        return np.ascontiguousarray(
            x16.reshape(ncores, mc, P, F)[:, m0 : m0 + nm]
            .transpose(2, 0, 1, 3)
            .reshape(P, ncores * nm * F)
        )

    xtb = np.ascontiguousarray(
        np.concatenate([part_x(m0, nm) for (m0, nm) in PARTS], axis=1)
    )

    jorder = np.asarray(_jorder())
    in_maps = []
    for c in range(ncores):
        rows = slice(c * rpc, (c + 1) * rpc)
        gt_c = g16[rows, :].T  # [np_total, rpc] bf16 view
        m = {"xtb": xtb, "wp": wp}
        for i, (s, l) in enumerate(FCHUNKS):
            # partition-major supertile layout in jorder
            chunk = np.ascontiguousarray(gt_c[:, s : s + l]).reshape(jc, P, l)
            m[f"g{i}"] = np.ascontiguousarray(
                chunk[jorder].transpose(1, 0, 2).reshape(P, jcv * l)
            )
        in_maps.append(m)
    return in_maps


def assemble_output(results, x, weight, n=N, ncores=NCORES):
    out_t = sum(
        np.concatenate(
            [results[c][f"out{k}T"] for c in range(ncores)], axis=1
        ).astype(np.float32)
        for k in range(1, ORDER)
    )
    out = np.ascontiguousarray(out_t.T[:n])
    # k=0 term, host-side in fp32
    wp0 = (weight[0] - weight[2]).astype(np.float32)
    out += np.asarray(x, dtype=np.float32) @ wp0
    return out


def kernel(x, gso, weight):
    import time

    from concourse import bass_utils

    nc = get_nc()
    in_maps = prep_inputs(x, gso, weight)
    last_err = None
    for attempt in range(3):
        try:
            res = bass_utils.run_bass_kernel_spmd(
                nc, in_maps, core_ids=list(range(NCORES))
            )
            return assemble_output(res.results, x, weight)
        except Exception as e:  # transient device wedge: retry
            last_err = e
            time.sleep(5.0 * (attempt + 1))
    raise last_err


# revision 66
# speedup vs baseline: 1.1930x; 1.0834x over previous
"""ChebConv (order-4) GNN layer on 8 Trainium2 NeuronCores.

Reference computation (fp32):
    T0 = x, T1 = G x, Tk = 2 G T{k-1} - T{k-2}
    out = sum_k Tk @ W[k]          # [N, F] with N=10000, F=32

Strategy:
  * Rewrite in the power basis: y0 = x, yk = G y{k-1},
      out = sum_k yk @ Wp[k]  with
      Wp = [W0 - W2, W1 - 3 W3, 2 W2, 4 W3]   (exact modulo fp reassociation)
    so each hop is a bare matmul against G (no 2*/- epilogue).
  * Row-shard G over 8 cores (1280 padded rows each; pad N 10000 ->
    10240).  Everything runs in plain bf16 (fp32 PSUM accumulation):
    measured end-to-end relative error ~3.4e-3 against the fp64
    oracle, inside the 2e-2 gate, and one bf16 pass per hop is 3x
    less PE streaming and 2x less HBM than an fp32-accurate scheme.
  * The ENTIRE per-core G^T block (~202KB/partition bf16) is pinned
    in SBUF: G is read from HBM exactly once, during hop 1.  Hops 2
    and 3 run pure PE with zero G DMA, which also keeps the shared
    HWDGE completion-semaphore rotation free of slow epochs (those
    stalled the collective staging DMAs by 30-50us in streaming
    variants).  To fit, everything non-G is trimmed to ~13KB per
    partition: y^T lives per-sweep in bf16, the k=0 term (x @ Wp0)
    is added by the HOST after gather, per-hop output partials ship
    to DRAM in bf16 for host summation (no on-chip cross-hop
    accumulator; PSUM groups held open across hops wedge the PE),
    and part 2's v tile is reused in place (its gather completes at
    hop end anyway).
  * Each hop runs 3 sweeps over column chunks (512, 384, 384) of
    yk^T.  Per sweep: matmuls (lhsT=v[j-chunk] [128,32], rhs=G^T
    tile [128,l]) accumulate the sweep's [32,l] chunk of yk^T over
    all 79 valid 128-row j-chunks in one open PSUM accumulation
    group (the all-padding j-chunk 79 is skipped).
  * The host pre-lays G^T out partition-major in the kernel's j
    consumption order: g_i[p, pos*l + c] = G^T[jorder[pos]*128 + p,
    s + c].  Hop-1 fills are plain 2D DMAs with 6-8KB contiguous
    per-partition lines (8 j-chunks per descriptor), near peak DMA
    rate, consumed incrementally by the PE.
  * After each sweep (except in the last hop), its rows are
    PE-transposed into natural m-chunk layout, cast bf16, and
    all-gathered in a partial collective (DRAM bounce) that overlaps
    the remaining sweeps.  Queue placement is load-bearing: cc_in
    staging rides scalar (HWDGE; its rotation holds only fast
    epochs), v reloads ride sync (idle after hop 1), and the output
    partials ride SWDGE so the y_t bounce never waits on the HWDGE
    rotation.  j-chunks are consumed in gather-firing order so each
    hop starts on columns whose gather finished first.  The
    remaining stalls (~25us) are the collectives stream's cold-start
    barrier (33-55us, runtime-internal) gating the first gather, and
    ~8-18us per collective op thereafter.
  * Output: three per-hop partials [32, 1280] bf16 per core; the
    host concatenates, transposes, sums, drops padding and adds the
    k=0 term in fp32.
"""

import sys

if "/opt/trn_rl_repo" not in sys.path:
    sys.path.insert(0, "/opt/trn_rl_repo")

import numpy as np

N = 10000
F = 32
ORDER = 4
NCORES = 8
P = 128
NP = 10240  # padded node count: divisible by NCORES * P
RPC = NP // NCORES  # rows per core (1280)
JC = NP // P  # global 128-row chunks (80); the last is all padding
JCV = JC - 1  # valid chunks
MC = RPC // P  # local 128-row chunks per core (10)

# column chunks of the per-core output slice: (start, len)
FCHUNKS = [(0, 512), (512, 384), (896, 384)]
# per-part m-chunk geometry: part i covers m-chunks [m0, m0+nm)
PARTS = [(0, 4), (4, 3), (7, 3)]
SWEEP_ORDER = [0, 1, 2]
GRP = 8  # j-chunks per hop-1 fill descriptor

_CACHE = {}


def _jorder():
    """j-chunk consumption order: grouped by producing part in sweep
    order (= gather firing order), skipping the all-padding chunk."""
    order = []
    for i in SWEEP_ORDER:
        m0, nm = PARTS[i]
        for c in range(NCORES):
            for mm in range(nm):
                j = c * MC + m0 + mm
                if j != JC - 1:
                    order.append(j)
    return order


def _build(np_total, ncores):
    from concourse import bacc, masks, mybir, tile

    rpc = np_total // ncores
    jc = np_total // P
    mc = rpc // P
    f32 = mybir.dt.float32
    bf16 = mybir.dt.bfloat16
    jcv = jc - 1

    nc = bacc.Bacc(
        "TRN2", target_bir_lowering=False, debug=False, num_devices=ncores
    )
    # per-sweep G^T blocks, host-laid-out partition-major in jorder:
    # g_i[p, pos*l + c] = G^T[jorder[pos]*P + p, s + c]
    gs = [
        nc.dram_tensor(f"g{i}", [P, jcv * l], bf16, kind="ExternalInput").ap()
        for i, (s, l) in enumerate(FCHUNKS)
    ]
    vcols = [ncores * nm * F for (m0, nm) in PARTS]
    # x in per-part v layout: block col (c*nm + ml)*F + f
    #   = padded x row (c*mc + m0 + ml)*P + p
    xtb = nc.dram_tensor("xtb", [P, sum(vcols)], bf16, kind="ExternalInput").ap()
    wp = nc.dram_tensor("wp", [F, (ORDER - 1) * F], bf16, kind="ExternalInput").ap()
    # per-hop partial outputs (y_k @ Wp_k)^T; the host sums them
    outs_t = [
        nc.dram_tensor(f"out{k}T", [F, rpc], bf16, kind="ExternalOutput").ap()
        for k in range(1, ORDER)
    ]

    jorder = _jorder()
    assert len(jorder) == jcv
    groups = [(g0, min(GRP, jcv - g0)) for g0 in range(0, jcv, GRP)]

    def part_of(m):
        for i, (m0, nm) in enumerate(PARTS):
            if m0 <= m < m0 + nm:
                return i
        raise AssertionError

    with tile.TileContext(nc) as tc:
        with (
            tc.tile_pool(name="const", bufs=1) as constp,
            tc.tile_pool(name="vp", bufs=2) as vp,
            tc.tile_pool(name="sb", bufs=1) as sb,
            tc.tile_pool(name="ps_hop", bufs=1, space="PSUM") as ps_hop,
            tc.tile_pool(name="ps_tp", bufs=2, space="PSUM") as ps_tp,
            tc.tile_pool(name="ps_w", bufs=2, space="PSUM") as ps_w,
            tc.tile_pool(name="dram", bufs=2, space="DRAM") as dram,
        ):
            ident = constp.tile([F, F], bf16)
            masks.make_identity(nc, ident[:])
            w_sb = constp.tile([F, (ORDER - 1) * F], bf16)
            nc.scalar.dma_start(w_sb[:], wp)
            pins = [
                constp.tile([P, jcv * l], bf16, name=f"pin{i}")
                for i, (s, l) in enumerate(FCHUNKS)
            ]
            # all hop-1 G fills issue up front so the sync queue never
            # stalls fill issue behind a cc_in staging wait; the PE's
            # matmuls consume each slice as its descriptor lands
            for i in SWEEP_ORDER:
                l = FCHUNKS[i][1]
                for g0, gl in groups:
                    nc.sync.dma_start(
                        pins[i][:, g0 * l : (g0 + gl) * l],
                        gs[i][:, g0 * l : (g0 + gl) * l],
                    )

            # v holds y_{k-1} in bf16, one tile per fc part so next-hop
            # matmuls only depend on the partial gather that produced
            # their columns
            v_parts = []
            off = 0
            for i, w_ in enumerate(vcols):
                if i < 2:
                    vt = vp.tile([P, w_], bf16, tag=f"v{i}", name=f"v{i}")
                else:
                    vt = constp.tile([P, w_], bf16, name=f"v{i}")
                nc.scalar.dma_start(vt[:], xtb[:, off : off + w_])
                off += w_
                v_parts.append(vt)

            def v_of(vps, j):
                c, m = j // mc, j % mc
                i = part_of(m)
                m0, nm = PARTS[i]
                col = (c * nm + (m - m0)) * F
                return vps[i][:, col : col + F]

            for k in range(1, ORDER):
                v_cur = v_parts
                if k < ORDER - 1:
                    # parts 0/1 double-buffer so their reloads can land
                    # mid-hop; part 2's gather only completes at hop end
                    # anyway, so its reload overwrites the tile in place
                    v_next = [
                        vp.tile([P, w_], bf16, tag=f"v{i}", name=f"vn{i}_{k}")
                        if i < 2
                        else v_parts[2]
                        for i, w_ in enumerate(vcols)
                    ]
                reloads = []
                # hop: y_k^T = (G @ y_{k-1})^T, one sweep per fc chunk so
                # partial all-gathers overlap the remaining sweeps
                for i in SWEEP_ORDER:
                    s, l = FCHUNKS[i]
                    y_t = sb.tile([F, l], bf16, tag="yT", name="yT")
                    hp = ps_hop.tile([F, l], f32, tag=f"hop{i}", name=f"hp{i}")
                    for pos in range(jcv):
                        nc.tensor.matmul(
                            hp[:],
                            lhsT=v_of(v_cur, jorder[pos]),
                            rhs=pins[i][:, pos * l : (pos + 1) * l],
                            start=(pos == 0),
                            stop=(pos == jcv - 1),
                        )
                    # sweep epilogue: copy out, Wp contribution
                    nc.vector.tensor_copy(y_t[:], hp[:])
                    pw = ps_w.tile([F, l], f32, tag="pw", name="pw")
                    nc.tensor.matmul(
                        pw[:], lhsT=w_sb[:, (k - 1) * F : k * F],
                        rhs=y_t[:], start=True, stop=True,
                    )
                    if k < ORDER - 1:
                        # transpose this sweep's rows to natural layout,
                        # cast bf16, partial all-gather
                        m0, nm = PARTS[i]
                        stage = sb.tile(
                            [P, 4 * F], bf16, tag="stage", name="stage"
                        )
                        for mm in range(nm):
                            tp = ps_tp.tile([P, F], bf16, tag="tp", name="tp")
                            nc.tensor.transpose(
                                tp[:], y_t[:, mm * P : (mm + 1) * P],
                                ident[:],
                            )
                            nc.vector.tensor_copy(
                                stage[:, mm * F : (mm + 1) * F], tp[:]
                            )
                        cc_in = dram.tile(
                            [P, nm * F], bf16, tag=f"ccin{i}",
                            name=f"ccin{i}",
                        )
                        cc_out = dram.tile(
                            [ncores * P, nm * F], bf16, tag=f"ccout{i}",
                            name=f"ccout{i}",
                        )
                        # scalar: an otherwise-idle HWDGE ring, so the
                        # staging transfer starts immediately (the sync
                        # ring would queue it behind all hop-1 G fills)
                        nc.scalar.dma_start(cc_in[:], stage[:, 0 : nm * F])
                        nc.gpsimd.collective_compute(
                            "AllGather",
                            mybir.AluOpType.bypass,
                            replica_groups=[list(range(ncores))],
                            ins=[cc_in.opt()],
                            outs=[cc_out.opt()],
                        )
                        reloads.append((i, cc_out))
                    # fold this hop's output partial back through y_t
                    # (free once the transposes are done), bf16, ship it
                    # on scalar HWDGE: it must NOT ride SWDGE, where it
                    # would queue behind pending collectives and stall
                    # the next sweep's y_t reuse
                    nc.vector.tensor_copy(y_t[:], pw[:])
                    nc.scalar.dma_start(outs_t[k - 1][:, s : s + l], y_t[:])
                # v reloads ride the sync HWDGE ring (idle after hop-1's
                # fills, fast transfers, no software-path convoy).  The
                # gpsimd queue carries ONLY collective triggers, so every
                # gather fires the moment its staging lands; emitting the
                # reloads at hop end keeps their gather-gated semaphore
                # epochs >= 9 emission slots away from any cc_in staging.
                for i, cc_out in reloads:
                    nc.sync.dma_start(
                        v_next[i][:].rearrange("p (c m) -> p c m", c=ncores),
                        cc_out[:].rearrange("(c p) m -> p c m", p=P),
                    )
                if k < ORDER - 1:
                    v_parts = v_next

    nc.compile()
    return nc


def get_nc(np_total=NP, ncores=NCORES):
    key = (np_total, ncores)
    if key not in _CACHE:
        _CACHE[key] = _build(np_total, ncores)
    return _CACHE[key]


def prep_inputs(x, gso, weight, np_total=NP, ncores=NCORES):
    """Host-side shard prep. Returns in_maps for run_bass_kernel_spmd."""
    import ml_dtypes

    bf = ml_dtypes.bfloat16
    n = x.shape[0]
    rpc = np_total // ncores
    jc = np_total // P
    jcv = jc - 1
    mc = rpc // P

    x = np.asarray(x, dtype=np.float32)
    gso = np.asarray(gso, dtype=np.float32)
    weight = np.asarray(weight, dtype=np.float32)

    # power-basis weights for k>=1; k=0 is added on the host
    wp = np.concatenate(
        [
            weight[1] - 3.0 * weight[3],
            2.0 * weight[2],
            4.0 * weight[3],
        ],
        axis=1,
    ).astype(bf)  # [F, (ORDER-1)*F]

    xpad = np.zeros((np_total, F), dtype=np.float32)
    xpad[:n] = x

    gpad = np.zeros((np_total, np_total), dtype=np.float32)
    gpad[:n, :n] = gso
    g16 = gpad.astype(bf)

    # x (bf16) in the per-part v layout
    x16 = xpad.astype(bf)

    def part_x(m0, nm):
        return np.ascontiguousarray(
            x16.reshape(ncores, mc, P, F)[:, m0 : m0 + nm]
            .transpose(2, 0, 1, 3)
            .reshape(P, ncores * nm * F)
        )

    xtb = np.ascontiguousarray(
        np.concatenate([part_x(m0, nm) for (m0, nm) in PARTS], axis=1)
    )

    jorder = np.asarray(_jorder())
    in_maps = []
    for c in range(ncores):
        rows = slice(c * rpc, (c + 1) * rpc)
        gt_c = g16[rows, :].T  # [np_total, rpc] bf16 view
        m = {"xtb": xtb, "wp": wp}
        for i, (s, l) in enumerate(FCHUNKS):
            # partition-major supertile layout in jorder
            chunk = np.ascontiguousarray(gt_c[:, s : s + l]).reshape(jc, P, l)
            m[f"g{i}"] = np.ascontiguousarray(
                chunk[jorder].transpose(1, 0, 2).reshape(P, jcv * l)
            )
        in_maps.append(m)
    return in_maps


def assemble_output(results, x, weight, n=N, ncores=NCORES):
    out_t = sum(
        np.concatenate(
            [results[c][f"out{k}T"] for c in range(ncores)], axis=1
        ).astype(np.float32)
        for k in range(1, ORDER)
    )
    out = np.ascontiguousarray(out_t.T[:n])
    # k=0 term, host-side in fp32
    wp0 = (weight[0] - weight[2]).astype(np.float32)
    out += np.asarray(x, dtype=np.float32) @ wp0
    return out


def kernel(x, gso, weight):
    import time

    from concourse import bass_utils

    nc = get_nc()
    in_maps = prep_inputs(x, gso, weight)
    last_err = None
    for attempt in range(3):
        try:
            res = bass_utils.run_bass_kernel_spmd(
                nc, in_maps, core_ids=list(range(NCORES))
            )
            return assemble_output(res.results, x, weight)
        except Exception as e:  # transient device wedge: retry
            last_err = e
            time.sleep(5.0 * (attempt + 1))
    raise last_err
